# revision 22
# baseline (speedup 1.0000x reference)
"""Masked multi-head attention (B=32, N=512, E=512, H=8) on 8 Trainium2 cores.

Sharding: data-parallel over batch (4 batches per core); weights and mask
replicated. All layout transforms are host-side numpy.

Per-core pipeline (per batch):
  q/k proj   fp8 DoubleRow 2-chains -> e-major fp8 [e, n], bias fused into
             the PSUM->SBUF cast (tensor_scalar_add)
  v proj     bf16 4-chains -> n-major [n, (h, dh|1)] bf16, ones col ->
             softmax denominator falls out of P@V
  scores     fp8, head pair per [128,1024] psum (heads at PE rows 0/64)
  exp        ACT, scale=1/8 fused, psum -> bf16 P
  mask       P *= adjT on DVE/GpSimd (bf16 2x mode, broadcast over head pair)
  P@V        bf16 4-chains into one [128, 4*65] psum per head; col 64 = denom
  normalize  reciprocal of 4 strided cols + one broadcast tensor_tensor
  oT         batched DMA transpose ([128,512] -> [128,4,128] per instr)
  out        oT.T @ WoT (bf16 4-chains) + bias, interleaved into the next
             batch's attention for pipelining
"""

import numpy as np

import concourse.bass as bass
import concourse.tile as tile
from concourse import bacc, mybir
import concourse.bass_utils as bass_utils

N_CORES = 8
B, N, E, H = 32, 512, 512, 8
DH = E // H  # 64
BPC = B // N_CORES  # batches per core
P = 128
NT = N // P
ET = E // P
FP32 = mybir.dt.float32
BF16 = mybir.dt.bfloat16
FP8 = mybir.dt.float8e4
AF = mybir.ActivationFunctionType
DR = mybir.MatmulPerfMode.DoubleRow
MUL = mybir.AluOpType.mult
ADD = mybir.AluOpType.add

# ktc indices whose mask multiply runs on gpsimd (rest on DVE)
POOL_MASK = (0, 2)


def build_nc(loop_iters=1):
    nc = bacc.Bacc("TRN2", target_bir_lowering=False, debug=False,
                   num_devices=N_CORES)

    xT_d = nc.dram_tensor("xT8", [BPC, P, ET, N], FP8, kind="ExternalInput")
    xTb_d = nc.dram_tensor("xTb", [BPC, P, ET, N], BF16,
                           kind="ExternalInput")
    wq_d = nc.dram_tensor("Wq2", [P, ET, E], FP8, kind="ExternalInput")
    wk_d = nc.dram_tensor("Wk2", [P, ET, E], FP8, kind="ExternalInput")
    wv_d = nc.dram_tensor("Wv2", [P, ET, E], BF16, kind="ExternalInput")
    wo_d = nc.dram_tensor("WoT", [P, ET, E], BF16, kind="ExternalInput")
    bq_d = nc.dram_tensor("bqT", [P, ET], FP32, kind="ExternalInput")
    bk_d = nc.dram_tensor("bkT", [P, ET], FP32, kind="ExternalInput")
    bv_d = nc.dram_tensor("bvB", [P, E], FP32, kind="ExternalInput")
    bo_d = nc.dram_tensor("boB", [P, E], FP32, kind="ExternalInput")
    adj_d = nc.dram_tensor("adjT", [P, NT, N], BF16, kind="ExternalInput")
    out_d = nc.dram_tensor("out", [BPC, N, E], FP32, kind="ExternalOutput")

    with tile.TileContext(nc) as tc:
        with (
            tc.tile_pool(name="persist", bufs=1) as persist,
            tc.tile_pool(name="xt", bufs=2) as xt_pool,
            tc.tile_pool(name="qk", bufs=2) as qk_pool,
            tc.tile_pool(name="vx", bufs=2) as vx_pool,
            tc.tile_pool(name="pt", bufs=3) as pt_pool,
            tc.tile_pool(name="osb", bufs=2) as o_pool,
            tc.tile_pool(name="otsb", bufs=2) as ot_pool,
            tc.tile_pool(name="outsb", bufs=3) as out_pool,
            tc.tile_pool(name="small", bufs=8) as small_pool,
            tc.tile_pool(name="ps_s", bufs=2, space="PSUM") as ps_s_pool,
            tc.tile_pool(name="ps_p", bufs=2, space="PSUM") as ps_p_pool,
            tc.tile_pool(name="ps_o", bufs=2, space="PSUM") as ps_o_pool,
        ):
            wq_sb = persist.tile([P, ET, E], FP8)
            nc.sync.dma_start(wq_sb[:], wq_d.ap())
            wk_sb = persist.tile([P, ET, E], FP8)
            nc.sync.dma_start(wk_sb[:], wk_d.ap())
            wv_sb = persist.tile([P, ET, E], BF16)
            nc.sync.dma_start(wv_sb[:], wv_d.ap())
            wo_sb = persist.tile([P, ET, E], BF16)
            nc.sync.dma_start(wo_sb[:], wo_d.ap())
            bq_sb = persist.tile([P, ET], FP32)
            nc.sync.dma_start(bq_sb[:], bq_d.ap())
            bk_sb = persist.tile([P, ET], FP32)
            nc.sync.dma_start(bk_sb[:], bk_d.ap())
            bv_sb = persist.tile([P, E], FP32)
            nc.sync.dma_start(bv_sb[:], bv_d.ap())
            bo_sb = persist.tile([P, E], FP32)
            nc.sync.dma_start(bo_sb[:], bo_d.ap())
            adj_sb = persist.tile([P, NT, N], BF16)
            nc.sync.dma_start(adj_sb[:], adj_d.ap())
            # block-diagonal padded q (one buffer per in-flight batch):
            # rows 0:64 hold the even head at cols 0:N, rows 64:128 the odd
            # head at cols N:2N; the zero blocks are written once here and
            # never touched again, so one full-128-row matmul per (pair,
            # key-chunk) yields both heads' scores with no PE row-base
            # switching.
            qt_pad0 = persist.tile([P, ET, 2 * N], FP8)
            nc.vector.memset(qt_pad0[:], 0.0)
            qt_pad1 = persist.tile([P, ET, 2 * N], FP8)
            nc.vector.memset(qt_pad1[:], 0.0)

            import contextlib
            loop_cm = (tc.For_i(0, loop_iters, 1) if loop_iters > 1
                       else contextlib.nullcontext())
            with loop_cm:
                body(nc, tc, locals())

    nc.compile()
    return nc


def body(nc, tc, env):
    (xT_d, xTb_d, out_d, wq_sb, wk_sb, wv_sb, wo_sb, bq_sb, bk_sb, bv_sb,
     bo_sb, adj_sb) = (env[k] for k in (
         "xT_d", "xTb_d", "out_d", "wq_sb", "wk_sb", "wv_sb", "wo_sb",
         "bq_sb", "bk_sb", "bv_sb", "bo_sb", "adj_sb"))
    qt_pads = (env["qt_pad0"], env["qt_pad1"])
    (xt_pool, qk_pool, vx_pool, pt_pool, o_pool, ot_pool, out_pool,
     small_pool, ps_s_pool, ps_p_pool, ps_o_pool) = (env[k] for k in (
         "xt_pool", "qk_pool", "vx_pool", "pt_pool", "o_pool", "ot_pool",
         "out_pool", "small_pool", "ps_s_pool", "ps_p_pool", "ps_o_pool"))

    pending = [None]

    def issue_trans(args):
        bprev, o_prev, otprev = args
        for nt in range(NT):
            nc.sync.dma_start_transpose(
                otprev[:, :, nt * P:(nt + 1) * P], o_prev[:, nt, :])

    def issue_final(args):
        bprev, o_prev, otprev = args
        for nt in range(NT):
            ps_f = ps_p_pool.tile([P, E], FP32, tag="psp")
            for et in range(ET):
                nc.tensor.matmul(
                    ps_f[:], otprev[:, et, nt * P:(nt + 1) * P],
                    wo_sb[:, et, :],
                    start=(et == 0), stop=(et == ET - 1))
            ob = out_pool.tile([P, E], FP32, tag="ob", name=f"ob_{bprev}_{nt}")
            nc.vector.tensor_tensor(ob[:], ps_f[:], bo_sb[:], op=ADD)
            nc.sync.dma_start(
                out_d.ap()[bprev, nt * P:(nt + 1) * P, :], ob[:])

    for b in range(BPC):
        xt = xt_pool.tile([P, ET, N], FP8, name=f"xt_{b}")
        nc.sync.dma_start(xt[:], xT_d.ap()[b])
        xtb = xt_pool.tile([P, ET, N], BF16, tag="xtb", name=f"xtb_{b}")
        nc.sync.dma_start(xtb[:], xTb_d.ap()[b])

        # ---- q/k projections, e-major fp8 (DoubleRow 2-chains)
        qt_pad = qt_pads[b % 2]
        ktl = qk_pool.tile([P, ET, N], FP8, tag="kt", name=f"kt_{b}")
        for t in range(ET):
            for w_sb, b_sb, dst in ((wq_sb, bq_sb, None), (wk_sb, bk_sb, ktl)):
                ps = ps_p_pool.tile([P, N], FP32, tag="psp")
                for m in range(2):
                    nc.tensor.matmul(
                        ps[:], w_sb[:, 2 * m:2 * m + 2, t * P:(t + 1) * P],
                        xt[:, 2 * m:2 * m + 2, :],
                        start=(m == 0), stop=(m == 1), perf_mode=DR)
                if dst is None:
                    nc.vector.tensor_scalar_add(
                        qt_pad[0:64, t, 0:N], ps[0:64], b_sb[0:64, t:t + 1])
                    nc.scalar.activation(
                        qt_pad[64:P, t, N:2 * N], ps[64:P], AF.Identity,
                        bias=b_sb[64:P, t:t + 1], scale=1.0)
                else:
                    nc.vector.tensor_scalar_add(dst[:, t, :], ps[:],
                                                b_sb[:, t:t + 1])

        # ---- v projection (bf16), n-major + ones column
        vx = vx_pool.tile([P, NT, H, DH + 1], BF16, name=f"vx_{b}")
        nc.vector.memset(vx[:, :, :, DH:DH + 1], 1.0)
        for nt in range(NT):
            ps = ps_p_pool.tile([P, E], FP32, tag="psp")
            for kc in range(ET):
                nc.tensor.matmul(
                    ps[:], xtb[:, kc, nt * P:(nt + 1) * P],
                    wv_sb[:, kc, :],
                    start=(kc == 0), stop=(kc == ET - 1))
            nc.vector.tensor_tensor(
                vx[:, nt, :, 0:DH],
                ps.rearrange("p (h d) -> p h d", h=H),
                bv_sb.rearrange("p (h d) -> p h d", h=H), op=ADD)

        o_sb = o_pool.tile([P, NT, E], BF16, name=f"o_{b}")
        ot = ot_pool.tile([P, ET, N], BF16, name=f"ot_{b}")

        def issue_scores(hp):
            pts = pt_pool.tile([P, NT, 2 * N], BF16, tag="pt",
                               name=f"pt_{b}_{hp}")
            for ktc in range(NT):
                ps_s = ps_s_pool.tile([P, 2 * N], FP32, tag="ss",
                                      name=f"ss_{b}_{hp}_{ktc}")
                for e in range(2):
                    nc.tensor.matmul(
                        ps_s[:, e * N:(e + 1) * N],
                        ktl[:, hp, ktc * P:(ktc + 1) * P],
                        qt_pad[:, hp, e * N:(e + 1) * N],
                        start=True, stop=True)
                nc.scalar.activation(pts[:, ktc, :], ps_s[:], AF.Exp,
                                     scale=0.125)
                eng = nc.gpsimd if ktc in POOL_MASK else nc.vector
                ptsv = pts.rearrange("p k (e n) -> p k e n", e=2)
                eng.tensor_tensor(
                    ptsv[:, ktc], ptsv[:, ktc],
                    adj_sb[:, ktc:ktc + 1, :].broadcast_to([P, 2, N]),
                    op=MUL)
            return pts

        def issue_o(hp, pts):
            for e in range(2):
                h = 2 * hp + e
                o_ps = ps_o_pool.tile([P, NT * (DH + 1)], FP32, tag="pso",
                                      name=f"pso_{b}_{h}")
                for qi in range(NT):
                    for ktc in range(NT):
                        nc.tensor.matmul(
                            o_ps[:, qi * (DH + 1):(qi + 1) * (DH + 1)],
                            pts[:, ktc,
                                e * N + qi * P:e * N + (qi + 1) * P],
                            vx[:, ktc, h, :],
                            start=(ktc == 0), stop=(ktc == NT - 1))
                rc = small_pool.tile([P, NT], FP32, tag="rc",
                                     name=f"rc_{b}_{h}")
                nc.vector.reciprocal(rc[:], o_ps[:, DH::DH + 1])
                nc.vector.tensor_tensor(
                    o_sb[:, :, h * DH:(h + 1) * DH],
                    o_ps.rearrange("p (q d) -> p q d", d=DH + 1)[:, :, 0:DH],
                    rc.rearrange("p (q o) -> p q o", o=1)
                      .broadcast_to([P, NT, DH]),
                    op=MUL)

        prev = None
        for hp in range(H // 2):
            cur = issue_scores(hp)
            if hp == 1 and pending[0] is not None:
                issue_trans(pending[0])
            if hp == 2 and pending[0] is not None:
                issue_final(pending[0])
                pending[0] = None
            if prev is not None:
                issue_o(hp - 1, prev)
            prev = cur
        issue_o(H // 2 - 1, prev)
        pending[0] = (b, o_sb, ot)

    issue_trans(pending[0])
    issue_final(pending[0])


_NC_CACHE = {}


def get_nc(loop_iters=1):
    if loop_iters not in _NC_CACHE:
        _NC_CACHE[loop_iters] = build_nc(loop_iters)
    return _NC_CACHE[loop_iters]


def prep_inputs(x, adj, Wq, Wk, Wv, bq, bk, bv, Wo, bo):
    """Host-side layout prep -> per-core input maps."""
    import ml_dtypes
    F8 = ml_dtypes.float8_e4m3fn
    BF = ml_dtypes.bfloat16

    x = np.asarray(x, dtype=np.float32)
    Wq = np.asarray(Wq, np.float32)
    Wk = np.asarray(Wk, np.float32)
    Wv = np.asarray(Wv, np.float32)
    Wo = np.asarray(Wo, np.float32)
    bq = np.asarray(bq, np.float32)
    bk = np.asarray(bk, np.float32)
    bv = np.asarray(bv, np.float32)
    bo = np.asarray(bo, np.float32)
    adj = np.asarray(adj)

    def wprep(W, dt):
        # [p, kc, e_out]: rows of W.T chunked along e_in
        return np.ascontiguousarray(
            W.T.reshape(ET, P, E).transpose(1, 0, 2).astype(dt))

    shared = {
        "Wq2": wprep(Wq, F8),
        "Wk2": wprep(Wk, F8),
        "Wv2": wprep(Wv, BF),
        "WoT": wprep(Wo, BF),
        "bqT": np.ascontiguousarray(bq.reshape(ET, P).T),
        "bkT": np.ascontiguousarray(bk.reshape(ET, P).T),
        "bvB": np.ascontiguousarray(np.broadcast_to(bv, (P, E))),
        "boB": np.ascontiguousarray(np.broadcast_to(bo, (P, E))),
        # adjT[p, kchunk, q] = adj[q, kchunk*128+p]
        "adjT": np.ascontiguousarray(
            adj.T.astype(np.float32).reshape(NT, P, N).transpose(1, 0, 2)
            .astype(BF)),
    }
    in_maps = []
    for c in range(N_CORES):
        xs = x[c * BPC:(c + 1) * BPC]
        m = dict(shared)
        xr = xs.transpose(0, 2, 1).reshape(BPC, ET, P, N).transpose(0, 2, 1, 3)
        m["xT8"] = np.ascontiguousarray(xr.astype(F8))
        m["xTb"] = np.ascontiguousarray(xr.astype(BF))
        in_maps.append(m)
    return in_maps


def kernel(**inputs):
    import os
    # this container lacks the axon NTFF hook; never attempt tracing
    os.environ.setdefault("BASS_NEVER_TRACE", "1")
    nc = get_nc()
    in_maps = prep_inputs(**inputs)
    res = bass_utils.run_bass_kernel_spmd(
        nc, in_maps, core_ids=list(range(N_CORES)))
    return np.concatenate([r["out"] for r in res.results], axis=0)


# ---------------------------------------------------------------------------
# Benchmarking helpers (not used by the grading path).
def _make_sharded_fn(nc):
    import jax
    from jax.sharding import Mesh, PartitionSpec, NamedSharding
    from jax.experimental.shard_map import shard_map
    from concourse import bass2jax

    bass2jax.install_neuronx_cc_hook()
    pid = nc.partition_id_tensor
    in_names, out_names, out_avals = [], [], []
    for alloc in nc.m.functions[0].allocations:
        if not isinstance(alloc, mybir.MemoryLocationSet):
            continue
        name = alloc.memorylocations[0].name
        if alloc.kind == "ExternalInput":
            if pid is None or name != pid.name:
                in_names.append(name)
        elif alloc.kind == "ExternalOutput":
            out_names.append(name)
            out_avals.append(jax.core.ShapedArray(
                tuple(alloc.tensor_shape), mybir.dt.np(alloc.dtype)))
    all_in_names = in_names + out_names
    if pid is not None:
        all_in_names.append(pid.name)

    def _body(*args):
        operands = list(args)
        if pid is not None:
            operands.append(bass2jax.partition_id_tensor())
        return tuple(bass2jax._bass_exec_p.bind(
            *operands,
            out_avals=tuple(out_avals),
            in_names=tuple(all_in_names),
            out_names=tuple(out_names),
            lowering_input_output_aliases=(),
            sim_require_finite=True,
            sim_require_nnan=True,
            nc=nc,
        ))

    devices = jax.devices()[:N_CORES]
    mesh = Mesh(np.asarray(devices), ("core",))
    spec = PartitionSpec("core")
    nin = len(in_names) + len(out_names)
    fn = jax.jit(
        shard_map(_body, mesh=mesh, in_specs=(spec,) * nin,
                  out_specs=(spec,) * len(out_names), check_rep=False),
        keep_unused=True,
    )
    return fn, in_names, out_names, out_avals, mesh, spec


def benchmark(inputs, r1=256, r2=1024, n_rep=10):
    """Interleaved two-point measurement: the ~80 ms axon dispatch overhead
    (and its drift) cancels in the difference; device time dominates both."""
    import time
    import jax
    from jax.sharding import NamedSharding

    in_maps = prep_inputs(**inputs)

    def setup(r):
        nc = get_nc(r)
        fn, in_names, out_names, out_avals, mesh, spec = _make_sharded_fn(nc)
        sh = NamedSharding(mesh, spec)
        args = []
        for name in in_names:
            args.append(jax.device_put(
                np.concatenate([m[name] for m in in_maps], axis=0), sh))
        for av in out_avals:
            args.append(jax.device_put(
                np.zeros((N_CORES * av.shape[0],) + av.shape[1:], av.dtype),
                sh))
        out = fn(*args)
        jax.block_until_ready(out)
        return fn, args

    f1, a1 = setup(r1)
    f2, a2 = setup(r2)
    t1s, t2s = [], []
    for _ in range(n_rep):
        t0 = time.perf_counter()
        jax.block_until_ready(f1(*a1))
        t1s.append(time.perf_counter() - t0)
        t0 = time.perf_counter()
        jax.block_until_ready(f2(*a2))
        t2s.append(time.perf_counter() - t0)
    return (min(t2s) - min(t1s)) * 1e9 / (r2 - r1)
